# revision 9
# baseline (speedup 1.0000x reference)
"""Multi-head self-attention (B=4, S=2048, E=1024, H=16) on 8 NeuronCores.

Sharding: batch (4) x head-group (2 groups of 8 heads), one (b, g) pair per
core.  Each core computes Q/K/V projections for its head group, attention,
and a partial output projection (row-parallel over Wo); the host sums the
two head-group partials per batch.

Layout strategy: the host feeds x transposed (xT = x.T, [E, S]) so every
matmul's contraction dim lands on SBUF partitions with no on-chip
transposes.  Scores are computed transposed (scoresT[k, q] = K @ Q^T per
head), softmax denominators come free via a concurrent ones-column matmul
(col-tiled at PE columns 64), and attn@V with V stationary directly yields
attnT[d, q] — exactly the lhsT the output projection needs.

All matmuls run in float32r (TF32-like, full PE rate at N>=512); measured
end-to-end relative error vs the fp32 reference is ~2e-4.
"""

import numpy as np

import concourse.bacc as bacc
import concourse.mybir as mybir
import concourse.tile as tile
from concourse.bass_utils import run_bass_kernel_spmd

B, S, E, H = 4, 2048, 1024, 16
GROUPS = 2                 # tensor-parallel head groups
HG = H // GROUPS           # heads per core
DH = E // H                # head dim
DG = HG * DH               # projected dim per core (512)
ET, DT, ST = E // 128, DG // 128, S // 128
QCH = 1024                 # q-chunk (psum tile free size, 2 banks)
NQC = S // QCH
NH = QCH // 512            # N=512 matmul halves per chunk
SCALE = 1.0 / np.sqrt(DH)

f32 = mybir.dt.float32
f32r = mybir.dt.float32r
FT = mybir.ActivationFunctionType

_CACHE = {}


def _body(nc, tc, xT, wq, wk, wv, wo, bqk, bv, bo, ones_d, out):
    with tc.tile_pool(name="pers", bufs=1) as pers, \
         tc.tile_pool(name="pp", bufs=1, space="PSUM") as pp:
        qt = pers.tile([128, DT, S], f32r)     # Q^T  [d, s]
        kt = pers.tile([128, DT, S], f32r)     # K^T  [d, s]
        vv = pers.tile([128, ST, HG, DH + 1], f32r)   # V [s, h, d|1] (ones col -> softmax denom)
        ones = pers.tile([128, 128], f32r)
        nc.sync.dma_start(out=ones, in_=ones_d.bitcast(f32r))
        nc.sync.dma_start(
            out=vv[:, :, :, DH:DH + 1],
            in_=ones_d.bitcast(f32r).rearrange("p (a b c) -> p a b c", a=ST, b=HG),
        )
        bqk_sb = pers.tile([128, 2 * DT], f32)
        nc.sync.dma_start(out=bqk_sb, in_=bqk)
        bv_sb = pers.tile([1, DG], f32r)
        nc.sync.dma_start(out=bv_sb, in_=bv.bitcast(f32r))
        bo_sb = pers.tile([1, E], f32r)
        nc.sync.dma_start(out=bo_sb, in_=bo.bitcast(f32r))

        # ---- Phase 1: projections (xT + weight streamed through a scratch pool)
        with tc.tile_pool(name="p1", bufs=1) as p1:
            xt = p1.tile([128, ET, S], f32r)
            nc.sync.dma_start(
                out=xt, in_=xT.bitcast(f32r).rearrange("(a p) s -> p a s", p=128)
            )
            # Q^T / K^T: out[d, s] accumulated over e;  lhsT = W, rhs = xT
            for ip, (wdram, dst) in enumerate(((wq, qt), (wk, kt))):
                wsb = p1.tile([128, ET, DG], f32r, tag="w")
                nc.sync.dma_start(
                    out=wsb,
                    in_=wdram.bitcast(f32r).rearrange("(a p) d -> p a d", p=128),
                )
                for m in range(DT):
                    for c in range(NQC):
                        ps = pp.tile([128, QCH], f32, tag="mm", bufs=2)
                        for k in range(ET):
                            for nn in range(NH):
                                nc.tensor.matmul(
                                    ps[:, nn * 512:(nn + 1) * 512],
                                    wsb[:, k, m * 128:(m + 1) * 128],
                                    xt[:, k, c * QCH + nn * 512:c * QCH + (nn + 1) * 512],
                                    start=(k == 0),
                                    stop=(k == ET - 1),
                                )
                        nc.scalar.activation(
                            out=dst[:, m, c * QCH:(c + 1) * QCH],
                            in_=ps,
                            func=FT.Identity,
                            bias=bqk_sb[:, ip * DT + m:ip * DT + m + 1],
                        )
            # V: natural [s, d];  lhsT = xT, rhs = Wv; bias via K=1 ones matmul
            wsb = p1.tile([128, ET, DG], f32r, tag="w")
            nc.sync.dma_start(
                out=wsb, in_=wv.bitcast(f32r).rearrange("(a p) d -> p a d", p=128)
            )
            for ms in range(ST):
                ps = pp.tile([128, QCH], f32, tag="mm", bufs=2)
                for k in range(ET):
                    nc.tensor.matmul(
                        ps[:, 0:DG],
                        xt[:, k, ms * 128:(ms + 1) * 128],
                        wsb[:, k, :],
                        start=(k == 0),
                        stop=False,
                    )
                nc.tensor.matmul(
                    ps[:, 0:DG], ones[0:1, :], bv_sb, start=False, stop=True
                )
                nc.vector.tensor_copy(
                    out=vv[:, ms, :, 0:DH],
                    in_=ps[:, 0:DG].rearrange("p (h d) -> p h d", h=HG),
                )

        # ---- Phase 2+3: attention and output projection
        with tc.tile_pool(name="p3", bufs=1) as p3:
            wo_sb = p3.tile([128, DT, E], f32r)
            nc.sync.dma_start(
                out=wo_sb, in_=wo.bitcast(f32r).rearrange("(a p) e -> p a e", p=128)
            )
            for c in range(NQC):
                at = [p3.tile([128, QCH], f32r, tag="attnT", bufs=2 * DT,
                              name=f"at{j}")
                      for j in range(DT)]
                for pr in range(HG // 2):   # head pairs share a d-tile -> PE row-pack
                    avs = [pp.tile([128, QCH], f32, tag="av", bufs=2,
                                   name=f"av{i}") for i in range(2)]
                    for kk in range(ST):
                        scs = [pp.tile([128, QCH], f32, tag="mm", bufs=2,
                                       name=f"sc{i}") for i in range(2)]
                        for nn in range(NH):
                            lo, hi = nn * 512, (nn + 1) * 512
                            for i, sc in enumerate(scs):
                                o = i * 64
                                nc.tensor.matmul(
                                    sc[:, lo:hi],
                                    kt[o:o + 64, pr, kk * 128:(kk + 1) * 128],
                                    qt[o:o + 64, pr, c * QCH + lo:c * QCH + hi],
                                    start=True,
                                    stop=True,
                                )
                        for i, (sc, av) in enumerate(zip(scs, avs)):
                            h = 2 * pr + i
                            ex = p3.tile([128, QCH], f32r, tag="expt", bufs=4)
                            nc.scalar.activation(
                                out=ex, in_=sc, func=FT.Exp, scale=SCALE
                            )
                            for nn in range(NH):
                                lo, hi = nn * 512, (nn + 1) * 512
                                nc.tensor.matmul(
                                    av[0:DH + 1, lo:hi],
                                    vv[:, kk, h],
                                    ex[:, lo:hi],
                                    start=(kk == 0),
                                    stop=(kk == ST - 1),
                                )
                    for i, av in enumerate(avs):
                        h = 2 * pr + i
                        rec = p3.tile([1, QCH], f32r, tag="rec", bufs=2)
                        with nc.allow_low_precision(reason="softmax denom"):
                            nc.vector.reciprocal(out=rec, in_=av[64:65, :])
                        bc = p3.tile([64, QCH], f32r, tag="bc", bufs=2)
                        nc.gpsimd.partition_broadcast(out_ap=bc, in_ap=rec)
                        o = (h % 2) * 64
                        nc.vector.tensor_mul(
                            at[h // 2][o:o + 64, :], av[0:64, :], bc
                        )
                # output projection for this q-chunk (rows c*QCH .. +QCH of out)
                for ms in range(QCH // 128):
                    po = pp.tile([128, E], f32, tag="mm", bufs=2)
                    for nn in range(E // 512):
                        lo, hi = nn * 512, (nn + 1) * 512
                        for j in range(DT):
                            nc.tensor.matmul(
                                po[:, lo:hi],
                                at[j][:, ms * 128:(ms + 1) * 128],
                                wo_sb[:, j, lo:hi],
                                start=(j == 0),
                                stop=False,
                            )
                        nc.tensor.matmul(
                            po[:, lo:hi], ones[0:1, :], bo_sb[:, lo:hi],
                            start=False, stop=True,
                        )
                    ou = p3.tile([128, E], f32, tag="out", bufs=3)
                    nc.vector.tensor_copy(out=ou, in_=po)
                    r0 = c * QCH + ms * 128
                    nc.sync.dma_start(out=out[r0:r0 + 128, :], in_=ou)


def _build():
    nc = bacc.Bacc("TRN2", target_bir_lowering=False, debug=False)
    xT = nc.dram_tensor("xT", [E, S], f32, kind="ExternalInput").ap()
    wq = nc.dram_tensor("wq", [E, DG], f32, kind="ExternalInput").ap()
    wk = nc.dram_tensor("wk", [E, DG], f32, kind="ExternalInput").ap()
    wv = nc.dram_tensor("wv", [E, DG], f32, kind="ExternalInput").ap()
    wo = nc.dram_tensor("wo", [DG, E], f32, kind="ExternalInput").ap()
    bqk = nc.dram_tensor("bqk", [128, 2 * DT], f32, kind="ExternalInput").ap()
    bv = nc.dram_tensor("bv", [1, DG], f32, kind="ExternalInput").ap()
    bo = nc.dram_tensor("bo", [1, E], f32, kind="ExternalInput").ap()
    ones_d = nc.dram_tensor("ones", [128, 128], f32, kind="ExternalInput").ap()
    out = nc.dram_tensor("out", [S, E], f32, kind="ExternalOutput").ap()
    with tile.TileContext(nc) as tc:
        _body(nc, tc, xT, wq, wk, wv, wo, bqk, bv, bo, ones_d, out)
    nc.compile()
    return nc


def _in_maps(inputs):
    x = np.asarray(inputs["inputs"], np.float32)
    maps = []
    for b in range(B):
        xT = np.ascontiguousarray(x[b].T)
        for g in range(GROUPS):
            sl = slice(g * DG, (g + 1) * DG)
            bq_g = np.asarray(inputs["bq"], np.float32)[sl]
            bk_g = np.asarray(inputs["bk"], np.float32)[sl]
            bqk = np.concatenate(
                [bq_g.reshape(DT, 128).T, bk_g.reshape(DT, 128).T], axis=1
            )
            maps.append({
                "xT": xT,
                "wq": np.ascontiguousarray(np.asarray(inputs["Wq"], np.float32)[:, sl]),
                "wk": np.ascontiguousarray(np.asarray(inputs["Wk"], np.float32)[:, sl]),
                "wv": np.ascontiguousarray(np.asarray(inputs["Wv"], np.float32)[:, sl]),
                "wo": np.ascontiguousarray(np.asarray(inputs["Wo"], np.float32)[sl, :]),
                "bqk": np.ascontiguousarray(bqk),
                "bv": np.asarray(inputs["bv"], np.float32)[sl].reshape(1, DG),
                "bo": np.asarray(inputs["bo"], np.float32).reshape(1, E),
                "ones": np.ones((128, 128), np.float32),
            })
    return maps


def kernel(**inputs) -> np.ndarray:
    if "nc" not in _CACHE:
        _CACHE["nc"] = _build()
    nc = _CACHE["nc"]
    res = run_bass_kernel_spmd(nc, _in_maps(inputs), core_ids=list(range(B * GROUPS)))
    out = np.zeros((B, S, E), np.float32)
    for b in range(B):
        out[b] = res.results[2 * b]["out"] + res.results[2 * b + 1]["out"]
    return out


if __name__ == "__main__":
    import jax
    rng = np.random.default_rng(0)
    demo = {
        "inputs": rng.standard_normal((B, S, E), dtype=np.float32),
        "Wq": rng.standard_normal((E, E), dtype=np.float32) / 32,
        "bq": np.zeros(E, np.float32),
        "Wk": rng.standard_normal((E, E), dtype=np.float32) / 32,
        "bk": np.zeros(E, np.float32),
        "Wv": rng.standard_normal((E, E), dtype=np.float32) / 32,
        "bv": np.zeros(E, np.float32),
        "Wo": rng.standard_normal((E, E), dtype=np.float32) / 32,
        "bo": np.zeros(E, np.float32),
    }
    got = kernel(**demo)
    print(got.shape, got.dtype)
